# revision 14
# baseline (speedup 1.0000x reference)
"""CP/PARAFAC bilinear regression kernel for Trainium2 (8 NeuronCores).

Computes y[n] = beta_0 + sum_{i,j} x[n,i,j] * w[i,j],  w = gamma^T @ alpha.

Data-parallel over the batch axis: each of the 8 cores gets 16384 rows of x.

The reduction is HBM-bandwidth-bound; the kernel keeps the DMA engines
streaming and does the arithmetic on the tensor engine (own SBUF ports):

- Host: x is cast to fp16 and laid out feature-planar per core:
  xt[c, k, n] = x_row[n, c*112 + k] — 4 chunks of 112 features on 112 SBUF
  partitions, rows contiguous per feature (no padding, clean big DMAs).
- Device: w = gamma^T @ alpha is computed in fp32 on the PE, cast to fp16.
  A short dense-bf16 warmup burst flips the PE HAM clock gate to 2.4 GHz
  before the real matmuls. Main loop: one DMA per row-group (ramped sizes
  so compute starts early); per 512-row window the PE runs 4 matmuls
  (stationary w chunk [112,1] fp16, moving x^T [112,512] fp16) accumulating
  y in PSUM [1,512] fp32. A per-window DVE tensor_scalar folds +beta_0
  into the y row; per-group DMAs stream the result out.

Accuracy: fp16 quantization of x (~2.3e-4) and w (~2.1e-4) only; all
accumulation is fp32 in PSUM.
"""

import numpy as np

N_TOTAL = 131072
N_CORES = 8
N_PER_CORE = N_TOTAL // N_CORES  # 16384
NG = 7
NA = 64
RANK = 64
D = NG * NA  # 448
NCH = 4  # feature chunks
CK = D // NCH  # 112 features per chunk
P = 128
WIN = 512  # rows per PSUM window (one bank: 512 f32)
GROUPS = [512, 512, 1024, 2048, 2048, 2048, 2048, 2048, 2048, 2048]
assert sum(GROUPS) == N_PER_CORE
N_WARMUP_MM = 10

_CACHE = {}


def _build():
    from concourse import bacc, mybir, tile

    f32 = mybir.dt.float32
    f16 = mybir.dt.float16
    bf16 = mybir.dt.bfloat16

    nc = bacc.Bacc("TRN2", target_bir_lowering=False, debug=False)

    xt_d = nc.dram_tensor("xt", [NCH, CK, N_PER_CORE], f16, kind="ExternalInput").ap()
    gamma_d = nc.dram_tensor("gamma", [RANK, NG], f32, kind="ExternalInput").ap()
    alpha_d = nc.dram_tensor("alpha", [RANK, NA], f32, kind="ExternalInput").ap()
    beta_d = nc.dram_tensor("beta", [1], f32, kind="ExternalInput").ap()
    y_d = nc.dram_tensor("y", [N_PER_CORE], f32, kind="ExternalOutput").ap()
    w_d = nc.dram_tensor("w_scratch", [D], f32).ap()

    add = mybir.AluOpType.add

    with tile.TileContext(nc) as tc:
        with (
            tc.tile_pool(name="const", bufs=1) as cpool,
            tc.tile_pool(name="xp", bufs=3) as xpool,
            tc.tile_pool(name="ps", bufs=8, space="PSUM") as pspool,
        ):
            # ---- PE HAM warmup: dense bf16 matmuls to lift the clock gate
            wj = cpool.tile([P, P], bf16)
            nc.vector.memset(wj[:], 1.0)
            xj = cpool.tile([P, WIN], bf16)
            nc.vector.memset(xj[:], 1.0)
            for _ in range(N_WARMUP_MM):
                pj = pspool.tile([P, WIN], f32, name="psw", tag="psw")
                nc.tensor.matmul(pj[:], wj[:], xj[:], start=True, stop=True)

            # ---- w = gamma^T @ alpha on the PE (K = RANK = 64), fp32
            g_sb = cpool.tile([RANK, NG], f32)
            a_sb = cpool.tile([RANK, NA], f32)
            nc.sync.dma_start(out=g_sb[:], in_=gamma_d[:, :])
            nc.sync.dma_start(out=a_sb[:], in_=alpha_d[:, :])
            # stage through DVE so the PE matmul needs only ONE sem wait
            ga_sb = cpool.tile([RANK, NG + NA], f32)
            nc.vector.tensor_copy(out=ga_sb[:, :NG], in_=g_sb[:])
            nc.vector.tensor_copy(out=ga_sb[:, NG:], in_=a_sb[:])
            w_ps = pspool.tile([NG, NA], f32, name="psw", tag="psw")
            nc.tensor.matmul(
                w_ps[:], ga_sb[:, :NG], ga_sb[:, NG:], start=True, stop=True
            )
            w_sb = cpool.tile([NG, NA], f32)
            nc.scalar.copy(out=w_sb[:], in_=w_ps[:])
            nc.sync.dma_start(out=w_d.rearrange("(i j) -> i j", i=NG), in_=w_sb[:])

            # ---- bounce w back as [112 feats, 4 chunks] (flat f = c*112 + k)
            w32 = cpool.tile([CK, NCH], f32)
            nc.sync.dma_start(
                out=w32[:], in_=w_d.rearrange("(c k) -> k c", c=NCH)
            )
            w16 = cpool.tile([CK, NCH], f16)
            nc.vector.tensor_copy(out=w16[:], in_=w32[:])

            beta_sb = cpool.tile([1, 1], f32)
            nc.sync.dma_start(out=beta_sb[:], in_=beta_d[None, :])

            y_row = cpool.tile([1, N_PER_CORE], f32)

            # ---- main loop
            row0 = 0
            for g, grows in enumerate(GROUPS):
                xt = xpool.tile([CK, NCH, grows], f16, name="xt", tag="xt")
                nc.sync.dma_start(
                    out=xt[:],
                    in_=xt_d[:, :, row0 : row0 + grows].rearrange("c k j -> k c j"),
                )
                nwin = grows // WIN
                # process in half-group batches of <=4 windows: c-major inside
                # a batch (amortizes LDW), folds right after each batch
                for b0 in range(0, nwin, 4):
                    bw = min(4, nwin - b0)
                    psums = [
                        pspool.tile([1, WIN], f32, name="psw", tag="psw")
                        for _ in range(bw)
                    ]
                    for c in range(NCH):
                        for w in range(bw):
                            j0 = (b0 + w) * WIN
                            nc.tensor.matmul(
                                psums[w][:],
                                w16[:, c : c + 1],
                                xt[:, c, j0 : j0 + WIN],
                                start=(c == 0),
                                stop=(c == NCH - 1),
                            )
                    for w in range(bw):
                        j0 = row0 + (b0 + w) * WIN
                        nc.vector.tensor_scalar(
                            out=y_row[:, j0 : j0 + WIN],
                            in0=psums[w][:],
                            scalar1=beta_sb[:],
                            scalar2=None,
                            op0=add,
                        )
                nc.sync.dma_start(
                    out=y_d[None, row0 : row0 + grows],
                    in_=y_row[:, row0 : row0 + grows],
                )
                row0 += grows

    nc.compile()
    return nc


def _prep_x(x):
    """Full x [131072, 7, 64] f32 -> per-core planar fp16 [4, 112, 16384]:
    xt[c, k, n] = x[core_base + n, flat=c*112+k]."""
    xf = np.asarray(x, dtype=np.float32).reshape(N_TOTAL, D)
    out = []
    for i in range(N_CORES):
        a = xf[i * N_PER_CORE : (i + 1) * N_PER_CORE]
        at = np.ascontiguousarray(a.T.astype(np.float16))
        out.append(at.reshape(NCH, CK, N_PER_CORE))
    return out


def _make_in_maps(x, beta_0, gamma, alpha):
    xt_shards = _prep_x(x)
    gamma_np = np.ascontiguousarray(np.asarray(gamma, dtype=np.float32))
    alpha_np = np.ascontiguousarray(np.asarray(alpha, dtype=np.float32))
    beta_np = np.asarray(beta_0, dtype=np.float32).reshape(1)
    return [
        {
            "xt": xt_shards[i],
            "gamma": gamma_np,
            "alpha": alpha_np,
            "beta": beta_np,
        }
        for i in range(N_CORES)
    ]


def kernel(x, beta_0, gamma, alpha):
    from concourse.bass_utils import run_bass_kernel_spmd

    if "nc" not in _CACHE:
        _CACHE["nc"] = _build()
    nc = _CACHE["nc"]

    in_maps = _make_in_maps(x, beta_0, gamma, alpha)
    res = run_bass_kernel_spmd(nc, in_maps, list(range(N_CORES)))
    y = np.concatenate([res.results[i]["y"] for i in range(N_CORES)])
    return y.astype(np.float32)


# revision 15
# speedup vs baseline: 1.3943x; 1.3943x over previous
"""CP/PARAFAC bilinear regression kernel for Trainium2 (8 NeuronCores).

Computes y[n] = beta_0 + sum_{i,j} x[n,i,j] * w[i,j],  w = gamma^T @ alpha.

Data-parallel over the batch axis: each of the 8 cores gets 16384 rows of x.

The reduction is HBM-bandwidth-bound (~15 MB/core at fp16), so the kernel
is organized to keep the DMA engines streaming continuously and hide all
arithmetic beneath them by splitting rows across two independent compute
paths (separate SBUF ports, no contention):

- PE path (rows 0..6143): host lays x out feature-planar fp16
  (xt[c, k, n] = x[n, c*112+k]); per 512-row window the tensor engine runs
  4 matmuls (stationary w chunk [112,1] fp16, moving x^T [112,512] fp16)
  accumulating in PSUM [1,512] fp32. The scalar engine (ACT) folds +beta_0
  into an SBUF y row and streams it out on its own HWDGE ring so the
  sync-ring x-load FIFO is never stalled.
- DVE path (rows 6144..16383): natural-layout fp16 row tiles; one fused
  scalar_tensor_tensor per 128-row block does multiply + row-sum in a
  single vector pass (accum_out), writing y columns for a contiguous
  final store. Row-to-partition mapping r = p*80 + t keeps that DMA clean.

w is computed on-device in fp32 on the PE and cast to fp16 (x's fp16
quantization ~2.3e-4 dominates the error; PSUM/accum stay fp32).
"""

import numpy as np

N_TOTAL = 131072
N_CORES = 8
N_PER_CORE = N_TOTAL // N_CORES  # 16384
NG = 7
NA = 64
RANK = 64
D = NG * NA  # 448
NCH = 4  # feature chunks (PE path)
CK = D // NCH  # 112
P = 128
WIN = 512  # rows per PSUM window

N_PE = 6144  # rows on the tensor-engine path
N_DVE = N_PER_CORE - N_PE  # 10240 rows on the vector-engine path
TD = N_DVE // P  # 80 y columns on the DVE path

PE_GROUPS = [512, 512, 1024, 2048, 2048]
DVE_TILES = [4, 12, 16, 16, 16, 16]
assert sum(PE_GROUPS) == N_PE and sum(DVE_TILES) == TD

_CACHE = {}


def _build():
    from concourse import bacc, mybir, tile

    f32 = mybir.dt.float32
    f16 = mybir.dt.float16

    nc = bacc.Bacc("TRN2", target_bir_lowering=False, debug=False)

    xt_d = nc.dram_tensor("xt", [NCH, CK, N_PE], f16, kind="ExternalInput").ap()
    xn_d = nc.dram_tensor("xn", [N_DVE, D], f16, kind="ExternalInput").ap()
    gamma_d = nc.dram_tensor("gamma", [RANK, NG], f32, kind="ExternalInput").ap()
    alpha_d = nc.dram_tensor("alpha", [RANK, NA], f32, kind="ExternalInput").ap()
    beta_d = nc.dram_tensor("beta", [1], f32, kind="ExternalInput").ap()
    y_d = nc.dram_tensor("y", [N_PER_CORE], f32, kind="ExternalOutput").ap()
    w_d = nc.dram_tensor("w_scratch", [D], f32).ap()

    add = mybir.AluOpType.add
    mult = mybir.AluOpType.mult

    with tile.TileContext(nc) as tc:
        with (
            tc.tile_pool(name="const", bufs=1) as cpool,
            tc.tile_pool(name="xpp", bufs=3) as xppool,
            tc.tile_pool(name="xpd", bufs=3) as xdpool,
            tc.tile_pool(name="sc", bufs=2) as scpool,
            tc.tile_pool(name="ps", bufs=8, space="PSUM") as pspool,
        ):
            # ---- w = gamma^T @ alpha on the PE (K = RANK = 64), fp32
            g_sb = cpool.tile([RANK, NG], f32)
            a_sb = cpool.tile([RANK, NA], f32)
            nc.sync.dma_start(out=g_sb[:], in_=gamma_d[:, :])
            nc.sync.dma_start(out=a_sb[:], in_=alpha_d[:, :])
            # stage through DVE so the PE matmul needs only ONE sem wait
            ga_sb = cpool.tile([RANK, NG + NA], f32)
            nc.vector.tensor_copy(out=ga_sb[:, :NG], in_=g_sb[:])
            nc.vector.tensor_copy(out=ga_sb[:, NG:], in_=a_sb[:])
            w_ps = pspool.tile([NG, NA], f32, name="psw", tag="psw")
            nc.tensor.matmul(
                w_ps[:], ga_sb[:, :NG], ga_sb[:, NG:], start=True, stop=True
            )
            w_sb = cpool.tile([NG, NA], f32)
            nc.scalar.copy(out=w_sb[:], in_=w_ps[:])
            nc.sync.dma_start(out=w_d.rearrange("(i j) -> i j", i=NG), in_=w_sb[:])

            # ---- PE-path weights: [112 feats, 4 chunks] fp16 (f = c*112 + k)
            w32 = cpool.tile([CK, NCH], f32)
            nc.sync.dma_start(out=w32[:], in_=w_d.rearrange("(c k) -> k c", c=NCH))
            w16 = cpool.tile([CK, NCH], f16)
            nc.vector.tensor_copy(out=w16[:], in_=w32[:])

            # ---- DVE-path weights: w broadcast to all partitions, fp16
            w_rep32 = cpool.tile([P, D], f32)
            nc.sync.dma_start(out=w_rep32[:], in_=w_d[None, :].to_broadcast((P, D)))
            w_rep16 = cpool.tile([P, D], f16)
            nc.vector.tensor_copy(out=w_rep16[:], in_=w_rep32[:])

            beta_sb = cpool.tile([1, 1], f32)
            nc.sync.dma_start(out=beta_sb[:], in_=beta_d[None, :])
            beta_bc = cpool.tile([P, 1], f32)
            nc.sync.dma_start(out=beta_bc[:], in_=beta_d[None, :].to_broadcast((P, 1)))

            y_row = cpool.tile([1, N_PE], f32)
            y_sb = cpool.tile([P, TD], f32)

            # DVE rows: partition p holds rows N_PE + p*TD + t
            xn_v = xn_d.rearrange("(p t) c -> p t c", p=P)

            # ---- interleaved main loop (PE groups / DVE tiles alternate so
            # both compute paths are fed early; all x loads on the sync ring)
            pe_row = 0
            dve_col = 0
            for step in range(max(len(PE_GROUPS), len(DVE_TILES))):
                if step < len(PE_GROUPS):
                    grows = PE_GROUPS[step]
                    xt = xppool.tile([CK, NCH, grows], f16, name="xt", tag="xt")
                    nc.sync.dma_start(
                        out=xt[:],
                        in_=xt_d[:, :, pe_row : pe_row + grows].rearrange(
                            "c k j -> k c j"
                        ),
                    )
                    nwin = grows // WIN
                    for b0 in range(0, nwin, 4):
                        bw = min(4, nwin - b0)
                        psums = [
                            pspool.tile([1, WIN], f32, name="psw", tag="psw")
                            for _ in range(bw)
                        ]
                        for c in range(NCH):
                            for w in range(bw):
                                j0 = (b0 + w) * WIN
                                nc.tensor.matmul(
                                    psums[w][:],
                                    w16[:, c : c + 1],
                                    xt[:, c, j0 : j0 + WIN],
                                    start=(c == 0),
                                    stop=(c == NCH - 1),
                                )
                        for w in range(bw):
                            j0 = pe_row + (b0 + w) * WIN
                            nc.scalar.add(
                                out=y_row[:, j0 : j0 + WIN],
                                in_=psums[w][:],
                                add=beta_sb[:],
                            )
                    nc.scalar.dma_start(
                        out=y_d[None, pe_row : pe_row + grows],
                        in_=y_row[:, pe_row : pe_row + grows],
                    )
                    pe_row += grows

                if step < len(DVE_TILES):
                    t_rows = DVE_TILES[step]
                    xn = xdpool.tile([P, t_rows, D], f16, name="xn", tag="xn")
                    nc.sync.dma_start(
                        out=xn[:], in_=xn_v[:, dve_col : dve_col + t_rows, :]
                    )
                    for k in range(t_rows):
                        sc = scpool.tile([P, D], f16, name="sc", tag="sc")
                        nc.vector.scalar_tensor_tensor(
                            out=sc[:],
                            in0=xn[:, k, :],
                            scalar=1.0,
                            in1=w_rep16[:],
                            op0=mult,
                            op1=mult,
                            accum_out=y_sb[:, dve_col + k : dve_col + k + 1],
                        )
                    dve_col += t_rows

            # ---- DVE epilogue: +beta, contiguous store of rows N_PE..end
            nc.vector.tensor_scalar_add(out=y_sb[:], in0=y_sb[:], scalar1=beta_bc[:])
            nc.scalar.dma_start(
                out=y_d[N_PE:].rearrange("(p t) -> p t", p=P), in_=y_sb[:]
            )

    nc.compile()
    return nc


def _prep_x(x):
    """Full x [131072,7,64] f32 -> per-core (planar fp16 [4,112,6144],
    natural fp16 [10240, 448])."""
    xf = np.asarray(x, dtype=np.float32).reshape(N_TOTAL, D)
    planar, natural = [], []
    for i in range(N_CORES):
        a = xf[i * N_PER_CORE : (i + 1) * N_PER_CORE]
        at = np.ascontiguousarray(a[:N_PE].T.astype(np.float16))
        planar.append(at.reshape(NCH, CK, N_PE))
        natural.append(a[N_PE:].astype(np.float16))
    return planar, natural


def _make_in_maps(x, beta_0, gamma, alpha):
    planar, natural = _prep_x(x)
    gamma_np = np.ascontiguousarray(np.asarray(gamma, dtype=np.float32))
    alpha_np = np.ascontiguousarray(np.asarray(alpha, dtype=np.float32))
    beta_np = np.asarray(beta_0, dtype=np.float32).reshape(1)
    return [
        {
            "xt": planar[i],
            "xn": natural[i],
            "gamma": gamma_np,
            "alpha": alpha_np,
            "beta": beta_np,
        }
        for i in range(N_CORES)
    ]


def kernel(x, beta_0, gamma, alpha):
    from concourse.bass_utils import run_bass_kernel_spmd

    if "nc" not in _CACHE:
        _CACHE["nc"] = _build()
    nc = _CACHE["nc"]

    in_maps = _make_in_maps(x, beta_0, gamma, alpha)
    res = run_bass_kernel_spmd(nc, in_maps, list(range(N_CORES)))
    y = np.concatenate([res.results[i]["y"] for i in range(N_CORES)])
    return y.astype(np.float32)
